# revision 29
# baseline (speedup 1.0000x reference)
"""BinaryLinear on 8 trn2 NeuronCores.

y = x @ sign(W).T + bias, x:(2,2048,4096) f32, W:(4096,4096) f32 [out,in],
bias:(4096,) f32.

Sharding: tensor-parallel over out_features — core c gets W rows
[c*512, (c+1)*512) and computes y[:, c*512:(c+1)*512] for all tokens.

Host marshalling (layout only — all of the module's arithmetic stays on
device): x is cast to bf16 and laid out transposed ([in, tokens]); W is
cast fp32->bf16 (sign-preserving — smallest |w| here is ~7e-8, far above
bf16 underflow) and laid out as the k-on-partition SBUF image
[pi, ko, n] per 128-out-feature chunk, so both matmul operands stream
from DRAM with plain full-bandwidth DMAs (no on-chip transposes needed).
Per-core outputs come back as y^T shards, re-assembled on the host.

Device kernel (per core):
  - W^T arrives in ko-quarters interleaved with the first x^T slices on
    the sync HWDGE queue (each load lands just before the matmul stream
    needs it); sign() runs on ScalarE per quarter, rotating across the
    4 out-feature chunks.
  - matmul stream: per 512-token group, the 4 psum banks (one per
    128-out-feature chunk) accumulate interleaved over ko-slices, so
    each x^T sub-load feeds 4x the PE work and the PE never outruns the
    DMA (lhsT = sign(W)^T chunk [128k x 128n], rhs = x^T block
    [128k x 512m], bf16 in / fp32 accum, 216 ns/matmul steady state).
  - bias added via ScalarE activation(Identity, bias=per-partition),
    fp32 y^T tiles DMA'd out on the ScalarE HWDGE queue.
Measured: ~251-253 us per core (the 1024 matmuls alone floor at
~221 us), vs 428 us for the first working version.
"""

import numpy as np
import ml_dtypes

B, S, D = 2, 2048, 4096
M = B * S            # 4096 tokens
NCORES = 8
NS = D // NCORES     # 512 out-features per core
P = 128
KO = D // P          # 32 contraction blocks
NC = NS // P         # 4 out-feature chunks per core
MB = 512             # tokens per matmul group (moving free dim)
MBL = 512            # tokens per x^T load chunk
HL = MBL // MB       # matmul-group halves per load chunk
MC = M // MBL        # 4 token load chunks
XSPLIT = 8           # x^T sub-loads per token chunk
KOS = KO // XSPLIT   # contraction blocks per sub-load

_CACHE = {}


def _build():
    import concourse.mybir as mybir
    import concourse.tile as tile
    from concourse import bacc
    from concourse.bass import ts

    nc = bacc.Bacc("TRN2", target_bir_lowering=False, debug=False)

    xt_d = nc.dram_tensor("xt_b", [D, M], mybir.dt.bfloat16, kind="ExternalInput")
    # wt_img[c, pi, ko, n] = bf16(W[c*128 + n, ko*128 + pi]) — SBUF image
    wt_img = nc.dram_tensor(
        "wt_img", [NC, P, KO, P], mybir.dt.bfloat16, kind="ExternalInput"
    )
    bias_pc = nc.dram_tensor("bias_pc", [P, NC], mybir.dt.float32, kind="ExternalInput")
    yt_d = nc.dram_tensor("yt", [NS, M], mybir.dt.float32, kind="ExternalOutput")

    # [D, M] viewed as [pi, ko, m] with k = ko*128 + pi
    xt_view = xt_d[:, :].rearrange("(ko pi) m -> pi ko m", pi=P)

    with tile.TileContext(nc) as tc:
        with (
            tc.tile_pool(name="const", bufs=1) as const_pool,
            tc.tile_pool(name="wt", bufs=1) as wt_pool,
            tc.tile_pool(name="xt", bufs=2) as xt_pool,
            tc.tile_pool(name="yt", bufs=2) as yt_pool,
            tc.tile_pool(name="psum", bufs=2, space="PSUM") as psum_pool,
        ):
            # wt_c[pi, ko, n] = sign(W_c[c*128 + n, ko*128 + pi])
            # wt is loaded in ko-quarters rotating across chunks, inter-
            # leaved on the sync queue with the first token chunk's x^T
            # sub-loads, so the first matmuls' exact dependencies (all
            # chunks' low-ko quarters + x slice 0) land first. Signs
            # follow the same rotation on ScalarE.
            # PE warm-up: ~12 dummy matmuls on zeroed SBUF fill the
            # otherwise-idle PE window during the input DMAs, so the HAM
            # clock gate is already at 2.4 GHz (8/8) when the real matmul
            # stream starts (first ~9 matmuls otherwise run at 1.2 GHz).
            warm = const_pool.tile([P, MB], mybir.dt.bfloat16)
            nc.gpsimd.memset(warm[:], 0)
            warm_ps = psum_pool.tile(
                [P, MB], mybir.dt.float32, tag="ps0", name="warm_ps"
            )
            NWARM = 24
            for i in range(NWARM):
                nc.tensor.matmul(
                    warm_ps[:], warm[:, :P], warm[:],
                    start=(i == 0), stop=(i == NWARM - 1),
                )

            NQ = 4
            QK = KO // NQ
            wts = [
                wt_pool.tile([P, KO, P], mybir.dt.bfloat16, name=f"wt{c}")
                for c in range(NC)
            ]
            xs0 = [
                xt_pool.tile(
                    [P, KOS, MBL], mybir.dt.bfloat16,
                    tag=f"xt{s}", name=f"xt{s}_0",
                )
                for s in range(XSPLIT)
            ]
            # sync-queue order: x slice 0, wt q0 (all chunks), x slice 1,
            # wt q1, x slice 2, wt q2, x slice 3, wt q3, x slices 4-7.
            def _load_wt_q(q):
                for c in range(NC):
                    nc.sync.dma_start(
                        wts[c][:, ts(q, QK), :], wt_img[c][:, ts(q, QK), :]
                    )

            def _load_x0(s):
                nc.sync.dma_start(xs0[s][:], xt_view[:, ts(s, KOS), ts(0, MBL)])

            _load_x0(0)
            _load_wt_q(0)
            _load_x0(1)
            _load_wt_q(1)
            _load_x0(2)
            _load_wt_q(2)
            _load_x0(3)
            _load_wt_q(3)
            for s in range(4, XSPLIT):
                _load_x0(s)
            for q in range(NQ):
                for c in range(NC):
                    sl = wts[c][:, ts(q, QK), :]
                    nc.scalar.activation(
                        sl, sl, mybir.ActivationFunctionType.Sign
                    )

            bias_sb = const_pool.tile([P, NC], mybir.dt.float32)
            nc.gpsimd.dma_start(bias_sb[:], bias_pc[:, :])

            for mc in range(MC):
                if mc == 0:
                    xs = xs0
                else:
                    xs = []
                    for s in range(XSPLIT):
                        xt_s = xt_pool.tile(
                            [P, KOS, MBL], mybir.dt.bfloat16, tag=f"xt{s}"
                        )
                        nc.sync.dma_start(
                            xt_s[:], xt_view[:, ts(s, KOS), ts(mc, MBL)]
                        )
                        xs.append(xt_s)

                # Interleave the 4 psum groups over ko-slices: each x^T
                # sub-load is consumed by all 4 out-feature chunks before
                # the next one is needed, so the PE never outruns the DMA.
                for h in range(HL):
                    pss = [
                        psum_pool.tile(
                            [P, MB], mybir.dt.float32,
                            tag=f"ps{c}", name=f"ps{c}_{mc}_{h}",
                        )
                        for c in range(NC)
                    ]
                    for s in range(XSPLIT):
                        for c in range(NC):
                            for kk in range(KOS):
                                ko = s * KOS + kk
                                nc.tensor.matmul(
                                    pss[c][:],
                                    wts[c][:, ko, :],
                                    xs[s][:, kk, ts(h, MB)],
                                    start=(ko == 0),
                                    stop=(ko == KO - 1),
                                )
                    for c in range(NC):
                        yt = yt_pool.tile(
                            [P, MB], mybir.dt.float32,
                            tag=f"yt{c}", name=f"yt{c}_{mc}_{h}",
                        )
                        nc.scalar.activation(
                            yt[:],
                            pss[c][:],
                            mybir.ActivationFunctionType.Identity,
                            bias=bias_sb[:, c : c + 1],
                        )
                        nc.scalar.dma_start(
                            yt_d[ts(c, P), ts(mc * HL + h, MB)], yt[:]
                        )

    nc.compile()
    return nc


def _run(inputs, trace=False, **spmd_kwargs):
    from concourse.bass_utils import run_bass_kernel_spmd

    x = np.asarray(inputs["x"], dtype=np.float32).reshape(M, D)
    weight = np.asarray(inputs["weight"], dtype=np.float32)
    bias = np.asarray(inputs["bias"], dtype=np.float32)

    xt_b = np.ascontiguousarray(x.T.astype(ml_dtypes.bfloat16))
    w_bf = weight.astype(ml_dtypes.bfloat16)
    in_maps = []
    for c in range(NCORES):
        # [NS, D] -> SBUF image [nc_chunk, pi, ko, n]
        w_c = w_bf[c * NS:(c + 1) * NS]
        wt_img = np.ascontiguousarray(
            w_c.reshape(NC, P, KO, P).transpose(0, 3, 2, 1)
        )
        b_pc = np.ascontiguousarray(
            bias[c * NS:(c + 1) * NS].reshape(NC, P).T
        )
        in_maps.append({"xt_b": xt_b, "wt_img": wt_img, "bias_pc": b_pc})

    if "nc" not in _CACHE:
        _CACHE["nc"] = _build()
    nc = _CACHE["nc"]

    res = run_bass_kernel_spmd(
        nc, in_maps, core_ids=list(range(NCORES)), trace=trace, **spmd_kwargs
    )
    # results[c]["yt"] is y[:, c*NS:(c+1)*NS].T — stack to y.T then transpose
    y_t = np.concatenate([res.results[c]["yt"] for c in range(NCORES)], axis=0)
    out = np.ascontiguousarray(y_t.T).reshape(B, S, D)
    return out, res


def kernel(**inputs) -> np.ndarray:
    out, _ = _run(inputs)
    return out


# revision 30
# speedup vs baseline: 1.0128x; 1.0128x over previous
"""BinaryLinear on 8 trn2 NeuronCores.

y = x @ sign(W).T + bias, x:(2,2048,4096) f32, W:(4096,4096) f32 [out,in],
bias:(4096,) f32.

Sharding: tensor-parallel over out_features — core c gets W rows
[c*512, (c+1)*512) and computes y[:, c*512:(c+1)*512] for all tokens.

Host marshalling (layout only — all of the module's arithmetic stays on
device): x is cast to bf16 and laid out transposed ([in, tokens]); W is
cast fp32->bf16 (sign-preserving — smallest |w| here is ~7e-8, far above
bf16 underflow) and laid out as the k-on-partition SBUF image
[pi, ko, n] per 128-out-feature chunk, so both matmul operands stream
from DRAM with plain full-bandwidth DMAs (no on-chip transposes needed).
Per-core outputs come back as y^T shards, re-assembled on the host.

Device kernel (per core):
  - W^T arrives in ko-quarters interleaved with the first x^T slices on
    the sync HWDGE queue (each load lands just before the matmul stream
    needs it); sign() runs on ScalarE per quarter, rotating across the
    4 out-feature chunks.
  - matmul stream: per 512-token group, the 4 psum banks (one per
    128-out-feature chunk) accumulate interleaved over ko-slices, so
    each x^T sub-load feeds 4x the PE work and the PE never outruns the
    DMA (lhsT = sign(W)^T chunk [128k x 128n], rhs = x^T block
    [128k x 512m], bf16 in / fp32 accum, 216 ns/matmul steady state).
  - bias added via ScalarE activation(Identity, bias=per-partition),
    fp32 y^T tiles DMA'd out on the ScalarE HWDGE queue.
A chain of ~24 dummy matmuls on zeroed SBUF bridges the input-DMA window
so the PE's HAM clock gate is already at 2.4 GHz when the real stream
starts (otherwise the first ~9 matmuls run at 1.2 GHz, and a warmup that
ends early gets re-throttled by the idle MID window).

Measured: ~249-253 us per core typical (occasional ~258-263 us run when
the chip hits the P0 power throttle); the 1024 matmuls alone floor at
~221 us, plus ~7 us Tile preamble, ~13 us end-of-kernel drain/barrier,
~14 us startup data staging. First working version was 428 us.
"""

import numpy as np
import ml_dtypes

B, S, D = 2, 2048, 4096
M = B * S            # 4096 tokens
NCORES = 8
NS = D // NCORES     # 512 out-features per core
P = 128
KO = D // P          # 32 contraction blocks
NC = NS // P         # 4 out-feature chunks per core
MB = 512             # tokens per matmul group (moving free dim)
MBL = 512            # tokens per x^T load chunk
HL = MBL // MB       # matmul-group halves per load chunk
MC = M // MBL        # 4 token load chunks
XSPLIT = 8           # x^T sub-loads per token chunk
KOS = KO // XSPLIT   # contraction blocks per sub-load

_CACHE = {}


def _build():
    import concourse.mybir as mybir
    import concourse.tile as tile
    from concourse import bacc
    from concourse.bass import ts

    nc = bacc.Bacc("TRN2", target_bir_lowering=False, debug=False)

    xt_d = nc.dram_tensor("xt_b", [D, M], mybir.dt.bfloat16, kind="ExternalInput")
    # wt_img[c, pi, ko, n] = bf16(W[c*128 + n, ko*128 + pi]) — SBUF image
    wt_img = nc.dram_tensor(
        "wt_img", [NC, P, KO, P], mybir.dt.bfloat16, kind="ExternalInput"
    )
    bias_pc = nc.dram_tensor("bias_pc", [P, NC], mybir.dt.float32, kind="ExternalInput")
    yt_d = nc.dram_tensor("yt", [NS, M], mybir.dt.float32, kind="ExternalOutput")

    # [D, M] viewed as [pi, ko, m] with k = ko*128 + pi
    xt_view = xt_d[:, :].rearrange("(ko pi) m -> pi ko m", pi=P)

    with tile.TileContext(nc) as tc:
        with (
            tc.tile_pool(name="const", bufs=1) as const_pool,
            tc.tile_pool(name="wt", bufs=1) as wt_pool,
            tc.tile_pool(name="xt", bufs=2) as xt_pool,
            tc.tile_pool(name="yt", bufs=2) as yt_pool,
            tc.tile_pool(name="psum", bufs=2, space="PSUM") as psum_pool,
        ):
            # wt_c[pi, ko, n] = sign(W_c[c*128 + n, ko*128 + pi])
            # wt is loaded in ko-quarters rotating across chunks, inter-
            # leaved on the sync queue with the first token chunk's x^T
            # sub-loads, so the first matmuls' exact dependencies (all
            # chunks' low-ko quarters + x slice 0) land first. Signs
            # follow the same rotation on ScalarE.
            # PE warm-up: ~12 dummy matmuls on zeroed SBUF fill the
            # otherwise-idle PE window during the input DMAs, so the HAM
            # clock gate is already at 2.4 GHz (8/8) when the real matmul
            # stream starts (first ~9 matmuls otherwise run at 1.2 GHz).
            warm = const_pool.tile([P, MB], mybir.dt.bfloat16)
            nc.gpsimd.memset(warm[:], 0)
            warm_ps = psum_pool.tile(
                [P, MB], mybir.dt.float32, tag="ps0", name="warm_ps"
            )
            NWARM = 24
            for i in range(NWARM):
                nc.tensor.matmul(
                    warm_ps[:], warm[:, :P], warm[:],
                    start=(i == 0), stop=(i == NWARM - 1),
                )

            NQ = 4
            QK = KO // NQ
            wts = [
                wt_pool.tile([P, KO, P], mybir.dt.bfloat16, name=f"wt{c}")
                for c in range(NC)
            ]
            xs0 = [
                xt_pool.tile(
                    [P, KOS, MBL], mybir.dt.bfloat16,
                    tag=f"xt{s}", name=f"xt{s}_0",
                )
                for s in range(XSPLIT)
            ]
            # sync-queue order: x slice 0, wt q0 (all chunks), x slice 1,
            # wt q1, x slice 2, wt q2, x slice 3, wt q3, x slices 4-7.
            def _load_wt_q(q):
                for c in range(NC):
                    nc.sync.dma_start(
                        wts[c][:, ts(q, QK), :], wt_img[c][:, ts(q, QK), :]
                    )

            def _load_x0(s):
                nc.sync.dma_start(xs0[s][:], xt_view[:, ts(s, KOS), ts(0, MBL)])

            _load_x0(0)
            _load_wt_q(0)
            _load_x0(1)
            _load_wt_q(1)
            _load_x0(2)
            _load_wt_q(2)
            _load_x0(3)
            _load_wt_q(3)
            for s in range(4, XSPLIT):
                _load_x0(s)
            for q in range(NQ):
                for c in range(NC):
                    sl = wts[c][:, ts(q, QK), :]
                    nc.scalar.activation(
                        sl, sl, mybir.ActivationFunctionType.Sign
                    )

            bias_sb = const_pool.tile([P, NC], mybir.dt.float32)
            nc.gpsimd.dma_start(bias_sb[:], bias_pc[:, :])

            for mc in range(MC):
                if mc == 0:
                    xs = xs0
                else:
                    xs = []
                    for s in range(XSPLIT):
                        xt_s = xt_pool.tile(
                            [P, KOS, MBL], mybir.dt.bfloat16, tag=f"xt{s}"
                        )
                        nc.sync.dma_start(
                            xt_s[:], xt_view[:, ts(s, KOS), ts(mc, MBL)]
                        )
                        xs.append(xt_s)

                # Interleave the 4 psum groups over ko-slices: each x^T
                # sub-load is consumed by all 4 out-feature chunks before
                # the next one is needed, so the PE never outruns the DMA.
                for h in range(HL):
                    pss = [
                        psum_pool.tile(
                            [P, MB], mybir.dt.float32,
                            tag=f"ps{c}", name=f"ps{c}_{mc}_{h}",
                        )
                        for c in range(NC)
                    ]
                    for s in range(XSPLIT):
                        for c in range(NC):
                            for kk in range(KOS):
                                ko = s * KOS + kk
                                nc.tensor.matmul(
                                    pss[c][:],
                                    wts[c][:, ko, :],
                                    xs[s][:, kk, ts(h, MB)],
                                    start=(ko == 0),
                                    stop=(ko == KO - 1),
                                )
                    for c in range(NC):
                        yt = yt_pool.tile(
                            [P, MB], mybir.dt.float32,
                            tag=f"yt{c}", name=f"yt{c}_{mc}_{h}",
                        )
                        nc.scalar.activation(
                            yt[:],
                            pss[c][:],
                            mybir.ActivationFunctionType.Identity,
                            bias=bias_sb[:, c : c + 1],
                        )
                        nc.scalar.dma_start(
                            yt_d[ts(c, P), ts(mc * HL + h, MB)], yt[:]
                        )

    nc.compile()
    return nc


def _run(inputs, trace=False, **spmd_kwargs):
    from concourse.bass_utils import run_bass_kernel_spmd

    x = np.asarray(inputs["x"], dtype=np.float32).reshape(M, D)
    weight = np.asarray(inputs["weight"], dtype=np.float32)
    bias = np.asarray(inputs["bias"], dtype=np.float32)

    xt_b = np.ascontiguousarray(x.T.astype(ml_dtypes.bfloat16))
    w_bf = weight.astype(ml_dtypes.bfloat16)
    in_maps = []
    for c in range(NCORES):
        # [NS, D] -> SBUF image [nc_chunk, pi, ko, n]
        w_c = w_bf[c * NS:(c + 1) * NS]
        wt_img = np.ascontiguousarray(
            w_c.reshape(NC, P, KO, P).transpose(0, 3, 2, 1)
        )
        b_pc = np.ascontiguousarray(
            bias[c * NS:(c + 1) * NS].reshape(NC, P).T
        )
        in_maps.append({"xt_b": xt_b, "wt_img": wt_img, "bias_pc": b_pc})

    if "nc" not in _CACHE:
        _CACHE["nc"] = _build()
    nc = _CACHE["nc"]

    res = run_bass_kernel_spmd(
        nc, in_maps, core_ids=list(range(NCORES)), trace=trace, **spmd_kwargs
    )
    # results[c]["yt"] is y[:, c*NS:(c+1)*NS].T — stack to y.T then transpose
    y_t = np.concatenate([res.results[c]["yt"] for c in range(NCORES)], axis=0)
    out = np.ascontiguousarray(y_t.T).reshape(B, S, D)
    return out, res


def kernel(**inputs) -> np.ndarray:
    out, _ = _run(inputs)
    return out


# revision 32
# speedup vs baseline: 1.0140x; 1.0012x over previous
"""BinaryLinear on 8 trn2 NeuronCores.

y = x @ sign(W).T + bias, x:(2,2048,4096) f32, W:(4096,4096) f32 [out,in],
bias:(4096,) f32.

Sharding: tensor-parallel over out_features — core c gets W rows
[c*512, (c+1)*512) and computes y[:, c*512:(c+1)*512] for all tokens.

Host marshalling (layout only — all of the module's arithmetic stays on
device): x is cast to bf16 and laid out transposed ([in, tokens]); W is
cast fp32->bf16 (sign-preserving — smallest |w| here is ~7e-8, far above
bf16 underflow) and laid out as the k-on-partition SBUF image
[pi, ko, n] per 128-out-feature chunk, so both matmul operands stream
from DRAM with plain full-bandwidth DMAs (no on-chip transposes needed).
Per-core outputs come back as y^T shards, re-assembled on the host.

Device kernel (per core):
  - W^T arrives in ko-quarters interleaved with the first x^T slices on
    the sync HWDGE queue (each load lands just before the matmul stream
    needs it); sign() runs on ScalarE per quarter, rotating across the
    4 out-feature chunks.
  - matmul stream: per 512-token group, the 4 psum banks (one per
    128-out-feature chunk) accumulate interleaved over ko-slices, so
    each x^T sub-load feeds 4x the PE work and the PE never outruns the
    DMA (lhsT = sign(W)^T chunk [128k x 128n], rhs = x^T block
    [128k x 512m], bf16 in / fp32 accum, 216 ns/matmul steady state).
  - bias added via ScalarE activation(Identity, bias=per-partition),
    fp32 y^T tiles DMA'd out on the ScalarE HWDGE queue.
A chain of ~24 dummy matmuls on zeroed SBUF bridges the input-DMA window
so the PE's HAM clock gate is already at 2.4 GHz when the real stream
starts (otherwise the first ~9 matmuls run at 1.2 GHz, and a warmup that
ends early gets re-throttled by the idle MID window).

Measured: ~249-253 us per core typical (occasional ~258-263 us run when
the chip hits the P0 power throttle); the 1024 matmuls alone floor at
~221 us, plus ~7 us Tile preamble, ~13 us end-of-kernel drain/barrier,
~14 us startup data staging. First working version was 428 us.
"""

import numpy as np
import ml_dtypes

B, S, D = 2, 2048, 4096
M = B * S            # 4096 tokens
NCORES = 8
NS = D // NCORES     # 512 out-features per core
P = 128
KO = D // P          # 32 contraction blocks
NC = NS // P         # 4 out-feature chunks per core
MB = 512             # tokens per matmul group (moving free dim)
MBL = 512            # tokens per x^T load chunk
HL = MBL // MB       # matmul-group halves per load chunk
MC = M // MBL        # 4 token load chunks
XSPLIT = 8           # x^T sub-loads per token chunk
KOS = KO // XSPLIT   # contraction blocks per sub-load

_CACHE = {}


def _build():
    import concourse.mybir as mybir
    import concourse.tile as tile
    from concourse import bacc
    from concourse.bass import ts

    nc = bacc.Bacc("TRN2", target_bir_lowering=False, debug=False)

    xt_d = nc.dram_tensor("xt_b", [D, M], mybir.dt.bfloat16, kind="ExternalInput")
    # wt_img[c, pi, ko, n] = bf16(W[c*128 + n, ko*128 + pi]) — SBUF image
    wt_img = nc.dram_tensor(
        "wt_img", [NC, P, KO, P], mybir.dt.bfloat16, kind="ExternalInput"
    )
    bias_pc = nc.dram_tensor("bias_pc", [P, NC], mybir.dt.float32, kind="ExternalInput")
    yt_d = nc.dram_tensor("yt", [NS, M], mybir.dt.float32, kind="ExternalOutput")

    # [D, M] viewed as [pi, ko, m] with k = ko*128 + pi
    xt_view = xt_d[:, :].rearrange("(ko pi) m -> pi ko m", pi=P)

    with tile.TileContext(nc) as tc:
        with (
            tc.tile_pool(name="const", bufs=1) as const_pool,
            tc.tile_pool(name="wt", bufs=1) as wt_pool,
            tc.tile_pool(name="xt", bufs=2) as xt_pool,
            tc.tile_pool(name="yt", bufs=2) as yt_pool,
            tc.tile_pool(name="psum", bufs=2, space="PSUM") as psum_pool,
        ):
            # wt_c[pi, ko, n] = sign(W_c[c*128 + n, ko*128 + pi])
            # wt is loaded in ko-quarters rotating across chunks, inter-
            # leaved on the sync queue with the first token chunk's x^T
            # sub-loads, so the first matmuls' exact dependencies (all
            # chunks' low-ko quarters + x slice 0) land first. Signs
            # follow the same rotation on ScalarE.
            # PE warm-up: ~12 dummy matmuls on zeroed SBUF fill the
            # otherwise-idle PE window during the input DMAs, so the HAM
            # clock gate is already at 2.4 GHz (8/8) when the real matmul
            # stream starts (first ~9 matmuls otherwise run at 1.2 GHz).
            warm = const_pool.tile([P, MB], mybir.dt.bfloat16)
            nc.gpsimd.memset(warm[:], 0)
            warm_ps = psum_pool.tile(
                [P, MB], mybir.dt.float32, tag="ps0", name="warm_ps"
            )
            NWARM = 20
            for i in range(NWARM):
                nc.tensor.matmul(
                    warm_ps[:], warm[:, :P], warm[:],
                    start=(i == 0), stop=(i == NWARM - 1),
                )

            NQ = 4
            QK = KO // NQ
            wts = [
                wt_pool.tile([P, KO, P], mybir.dt.bfloat16, name=f"wt{c}")
                for c in range(NC)
            ]
            xs0 = [
                xt_pool.tile(
                    [P, KOS, MBL], mybir.dt.bfloat16,
                    tag=f"xt{s}", name=f"xt{s}_0",
                )
                for s in range(XSPLIT)
            ]
            # sync-queue order: x slice 0, wt q0 (all chunks), x slice 1,
            # wt q1, x slice 2, wt q2, x slice 3, wt q3, x slices 4-7.
            def _load_wt_q(q):
                for c in range(NC):
                    nc.sync.dma_start(
                        wts[c][:, ts(q, QK), :], wt_img[c][:, ts(q, QK), :]
                    )

            def _load_x0(s):
                nc.sync.dma_start(xs0[s][:], xt_view[:, ts(s, KOS), ts(0, MBL)])

            def _load_wt_e(c, e):
                # eighth-granularity (4-ko / 128KB) wt load
                nc.sync.dma_start(
                    wts[c][:, ts(e, KOS), :], wt_img[c][:, ts(e, KOS), :]
                )

            # First wave at 128KB granularity: every chunk's ko0-3 block
            # (what matmul round s=0 needs) plus x slice 0 lead the queue.
            for c in range(NC):
                _load_wt_e(c, 0)
            _load_x0(0)
            for c in range(NC):
                _load_wt_e(c, 1)
            _load_x0(1)
            _load_wt_q(1)
            _load_x0(2)
            _load_wt_q(2)
            _load_x0(3)
            _load_wt_q(3)
            for s in range(4, XSPLIT):
                _load_x0(s)
            # signs: fine slices for ko0-7 (c-rotated), quarters after
            for e in range(2):
                for c in range(NC):
                    sl = wts[c][:, ts(e, KOS), :]
                    nc.scalar.activation(
                        sl, sl, mybir.ActivationFunctionType.Sign
                    )
            for q in range(1, NQ):
                for c in range(NC):
                    sl = wts[c][:, ts(q, QK), :]
                    nc.scalar.activation(
                        sl, sl, mybir.ActivationFunctionType.Sign
                    )

            bias_sb = const_pool.tile([P, NC], mybir.dt.float32)
            nc.gpsimd.dma_start(bias_sb[:], bias_pc[:, :])

            for mc in range(MC):
                if mc == 0:
                    xs = xs0
                else:
                    xs = []
                    for s in range(XSPLIT):
                        xt_s = xt_pool.tile(
                            [P, KOS, MBL], mybir.dt.bfloat16, tag=f"xt{s}"
                        )
                        nc.sync.dma_start(
                            xt_s[:], xt_view[:, ts(s, KOS), ts(mc, MBL)]
                        )
                        xs.append(xt_s)

                # Interleave the 4 psum groups over ko-slices: each x^T
                # sub-load is consumed by all 4 out-feature chunks before
                # the next one is needed, so the PE never outruns the DMA.
                for h in range(HL):
                    pss = [
                        psum_pool.tile(
                            [P, MB], mybir.dt.float32,
                            tag=f"ps{c}", name=f"ps{c}_{mc}_{h}",
                        )
                        for c in range(NC)
                    ]
                    for s in range(XSPLIT):
                        for c in range(NC):
                            for kk in range(KOS):
                                ko = s * KOS + kk
                                nc.tensor.matmul(
                                    pss[c][:],
                                    wts[c][:, ko, :],
                                    xs[s][:, kk, ts(h, MB)],
                                    start=(ko == 0),
                                    stop=(ko == KO - 1),
                                )
                    for c in range(NC):
                        yt = yt_pool.tile(
                            [P, MB], mybir.dt.float32,
                            tag=f"yt{c}", name=f"yt{c}_{mc}_{h}",
                        )
                        nc.scalar.activation(
                            yt[:],
                            pss[c][:],
                            mybir.ActivationFunctionType.Identity,
                            bias=bias_sb[:, c : c + 1],
                        )
                        nc.scalar.dma_start(
                            yt_d[ts(c, P), ts(mc * HL + h, MB)], yt[:]
                        )

    nc.compile()
    return nc


def _run(inputs, trace=False, **spmd_kwargs):
    from concourse.bass_utils import run_bass_kernel_spmd

    x = np.asarray(inputs["x"], dtype=np.float32).reshape(M, D)
    weight = np.asarray(inputs["weight"], dtype=np.float32)
    bias = np.asarray(inputs["bias"], dtype=np.float32)

    xt_b = np.ascontiguousarray(x.T.astype(ml_dtypes.bfloat16))
    w_bf = weight.astype(ml_dtypes.bfloat16)
    in_maps = []
    for c in range(NCORES):
        # [NS, D] -> SBUF image [nc_chunk, pi, ko, n]
        w_c = w_bf[c * NS:(c + 1) * NS]
        wt_img = np.ascontiguousarray(
            w_c.reshape(NC, P, KO, P).transpose(0, 3, 2, 1)
        )
        b_pc = np.ascontiguousarray(
            bias[c * NS:(c + 1) * NS].reshape(NC, P).T
        )
        in_maps.append({"xt_b": xt_b, "wt_img": wt_img, "bias_pc": b_pc})

    if "nc" not in _CACHE:
        _CACHE["nc"] = _build()
    nc = _CACHE["nc"]

    res = run_bass_kernel_spmd(
        nc, in_maps, core_ids=list(range(NCORES)), trace=trace, **spmd_kwargs
    )
    # results[c]["yt"] is y[:, c*NS:(c+1)*NS].T — stack to y.T then transpose
    y_t = np.concatenate([res.results[c]["yt"] for c in range(NCORES)], axis=0)
    out = np.ascontiguousarray(y_t.T).reshape(B, S, D)
    return out, res


def kernel(**inputs) -> np.ndarray:
    out, _ = _run(inputs)
    return out


# revision 33
# speedup vs baseline: 1.0368x; 1.0225x over previous
"""BinaryLinear on 8 trn2 NeuronCores.

y = x @ sign(W).T + bias, x:(2,2048,4096) f32, W:(4096,4096) f32 [out,in],
bias:(4096,) f32.

Sharding: tensor-parallel over out_features — core c gets W rows
[c*512, (c+1)*512) and computes y[:, c*512:(c+1)*512] for all tokens.

Host marshalling (layout only — all of the module's arithmetic stays on
device): x is cast to bf16 and laid out transposed ([in, tokens]); W is
cast fp32->bf16 (sign-preserving — smallest |w| here is ~7e-8, far above
bf16 underflow) and laid out as the k-on-partition SBUF image
[pi, ko, n] per 128-out-feature chunk, so both matmul operands stream
from DRAM with plain full-bandwidth DMAs (no on-chip transposes needed).
Per-core outputs come back as y^T shards, re-assembled on the host.

Device kernel (per core):
  - W^T arrives in ko-quarters interleaved with the first x^T slices on
    the sync HWDGE queue (each load lands just before the matmul stream
    needs it); sign() runs on ScalarE per quarter, rotating across the
    4 out-feature chunks.
  - matmul stream: per 512-token group, the 4 psum banks (one per
    128-out-feature chunk) accumulate interleaved over ko-slices, so
    each x^T sub-load feeds 4x the PE work and the PE never outruns the
    DMA (lhsT = sign(W)^T chunk [128k x 128n], rhs = x^T block
    [128k x 512m], bf16 in / fp32 accum, 216 ns/matmul steady state).
  - bias added via ScalarE activation(Identity, bias=per-partition),
    fp32 y^T tiles DMA'd out on the ScalarE HWDGE queue.
A chain of ~24 dummy matmuls on zeroed SBUF bridges the input-DMA window
so the PE's HAM clock gate is already at 2.4 GHz when the real stream
starts (otherwise the first ~9 matmuls run at 1.2 GHz, and a warmup that
ends early gets re-throttled by the idle MID window).

Measured: ~249-253 us per core typical (occasional ~258-263 us run when
the chip hits the P0 power throttle); the 1024 matmuls alone floor at
~221 us, plus ~7 us Tile preamble, ~13 us end-of-kernel drain/barrier,
~14 us startup data staging. First working version was 428 us.
"""

import numpy as np
import ml_dtypes

B, S, D = 2, 2048, 4096
M = B * S            # 4096 tokens
NCORES = 8
NS = D // NCORES     # 512 out-features per core
P = 128
KO = D // P          # 32 contraction blocks
NC = NS // P         # 4 out-feature chunks per core
MB = 512             # tokens per matmul group (moving free dim)
MBL = 512            # tokens per x^T load chunk
HL = MBL // MB       # matmul-group halves per load chunk
MC = M // MBL        # 4 token load chunks
XSPLIT = 8           # x^T sub-loads per token chunk
KOS = KO // XSPLIT   # contraction blocks per sub-load

_CACHE = {}


def _build():
    import concourse.mybir as mybir
    import concourse.tile as tile
    from concourse import bacc
    from concourse.bass import ts

    nc = bacc.Bacc("TRN2", target_bir_lowering=False, debug=False)

    xt_d = nc.dram_tensor("xt_b", [D, M], mybir.dt.bfloat16, kind="ExternalInput")
    # wt_img[c, pi, ko, n] = bf16(W[c*128 + n, ko*128 + pi]) — SBUF image
    wt_img = nc.dram_tensor(
        "wt_img", [NC, P, KO, P], mybir.dt.bfloat16, kind="ExternalInput"
    )
    bias_pc = nc.dram_tensor("bias_pc", [P, NC], mybir.dt.float32, kind="ExternalInput")
    yt_d = nc.dram_tensor("yt", [NS, M], mybir.dt.float32, kind="ExternalOutput")

    # [D, M] viewed as [pi, ko, m] with k = ko*128 + pi
    xt_view = xt_d[:, :].rearrange("(ko pi) m -> pi ko m", pi=P)

    with tile.TileContext(nc) as tc:
        with (
            tc.tile_pool(name="const", bufs=1) as const_pool,
            tc.tile_pool(name="wt", bufs=1) as wt_pool,
            tc.tile_pool(name="xt", bufs=2) as xt_pool,
            tc.tile_pool(name="yt", bufs=2) as yt_pool,
            tc.tile_pool(name="psum", bufs=2, space="PSUM") as psum_pool,
        ):
            # wt_c[pi, ko, n] = sign(W_c[c*128 + n, ko*128 + pi])
            # wt is loaded in ko-quarters rotating across chunks, inter-
            # leaved on the sync queue with the first token chunk's x^T
            # sub-loads, so the first matmuls' exact dependencies (all
            # chunks' low-ko quarters + x slice 0) land first. Signs
            # follow the same rotation on ScalarE.
            # PE warm-up: ~12 dummy matmuls on zeroed SBUF fill the
            # otherwise-idle PE window during the input DMAs, so the HAM
            # clock gate is already at 2.4 GHz (8/8) when the real matmul
            # stream starts (first ~9 matmuls otherwise run at 1.2 GHz).
            warm = const_pool.tile([P, MB], mybir.dt.bfloat16)
            nc.gpsimd.memset(warm[:], 0)
            warm_ps = psum_pool.tile(
                [P, MB], mybir.dt.float32, tag="ps0", name="warm_ps"
            )
            NWARM = 24
            for i in range(NWARM):
                nc.tensor.matmul(
                    warm_ps[:], warm[:, :P], warm[:],
                    start=(i == 0), stop=(i == NWARM - 1),
                )

            NQ = 4
            QK = KO // NQ
            wts = [
                wt_pool.tile([P, KO, P], mybir.dt.bfloat16, name=f"wt{c}")
                for c in range(NC)
            ]
            xs0 = [
                xt_pool.tile(
                    [P, KOS, MBL], mybir.dt.bfloat16,
                    tag=f"xt{s}", name=f"xt{s}_0",
                )
                for s in range(XSPLIT)
            ]
            # sync-queue order: x slice 0, wt q0 (all chunks), x slice 1,
            # wt q1, x slice 2, wt q2, x slice 3, wt q3, x slices 4-7.
            def _load_wt_q(q):
                for c in range(NC):
                    nc.sync.dma_start(
                        wts[c][:, ts(q, QK), :], wt_img[c][:, ts(q, QK), :]
                    )

            def _load_x0(s):
                nc.sync.dma_start(xs0[s][:], xt_view[:, ts(s, KOS), ts(0, MBL)])

            _load_x0(0)
            _load_wt_q(0)
            _load_x0(1)
            _load_wt_q(1)
            _load_x0(2)
            _load_wt_q(2)
            _load_x0(3)
            _load_wt_q(3)
            for s in range(4, XSPLIT):
                _load_x0(s)
            for q in range(NQ):
                for c in range(NC):
                    sl = wts[c][:, ts(q, QK), :]
                    nc.scalar.activation(
                        sl, sl, mybir.ActivationFunctionType.Sign
                    )

            bias_sb = const_pool.tile([P, NC], mybir.dt.float32)
            nc.gpsimd.dma_start(bias_sb[:], bias_pc[:, :])

            for mc in range(MC):
                if mc == 0:
                    xs = xs0
                else:
                    xs = []
                    for s in range(XSPLIT):
                        xt_s = xt_pool.tile(
                            [P, KOS, MBL], mybir.dt.bfloat16, tag=f"xt{s}"
                        )
                        nc.sync.dma_start(
                            xt_s[:], xt_view[:, ts(s, KOS), ts(mc, MBL)]
                        )
                        xs.append(xt_s)

                # Interleave the 4 psum groups over ko-slices: each x^T
                # sub-load is consumed by all 4 out-feature chunks before
                # the next one is needed, so the PE never outruns the DMA.
                for h in range(HL):
                    pss = [
                        psum_pool.tile(
                            [P, MB], mybir.dt.float32,
                            tag=f"ps{c}", name=f"ps{c}_{mc}_{h}",
                        )
                        for c in range(NC)
                    ]
                    for s in range(XSPLIT):
                        for c in range(NC):
                            for kk in range(KOS):
                                ko = s * KOS + kk
                                nc.tensor.matmul(
                                    pss[c][:],
                                    wts[c][:, ko, :],
                                    xs[s][:, kk, ts(h, MB)],
                                    start=(ko == 0),
                                    stop=(ko == KO - 1),
                                )
                    for c in range(NC):
                        yt = yt_pool.tile(
                            [P, MB], mybir.dt.float32,
                            tag=f"yt{c}", name=f"yt{c}_{mc}_{h}",
                        )
                        nc.scalar.activation(
                            yt[:],
                            pss[c][:],
                            mybir.ActivationFunctionType.Identity,
                            bias=bias_sb[:, c : c + 1],
                        )
                        nc.scalar.dma_start(
                            yt_d[ts(c, P), ts(mc * HL + h, MB)], yt[:]
                        )

    nc.compile()
    return nc


def _run(inputs, trace=False, **spmd_kwargs):
    from concourse.bass_utils import run_bass_kernel_spmd

    x = np.asarray(inputs["x"], dtype=np.float32).reshape(M, D)
    weight = np.asarray(inputs["weight"], dtype=np.float32)
    bias = np.asarray(inputs["bias"], dtype=np.float32)

    xt_b = np.ascontiguousarray(x.T.astype(ml_dtypes.bfloat16))
    w_bf = weight.astype(ml_dtypes.bfloat16)
    in_maps = []
    for c in range(NCORES):
        # [NS, D] -> SBUF image [nc_chunk, pi, ko, n]
        w_c = w_bf[c * NS:(c + 1) * NS]
        wt_img = np.ascontiguousarray(
            w_c.reshape(NC, P, KO, P).transpose(0, 3, 2, 1)
        )
        b_pc = np.ascontiguousarray(
            bias[c * NS:(c + 1) * NS].reshape(NC, P).T
        )
        in_maps.append({"xt_b": xt_b, "wt_img": wt_img, "bias_pc": b_pc})

    if "nc" not in _CACHE:
        _CACHE["nc"] = _build()
    nc = _CACHE["nc"]

    res = run_bass_kernel_spmd(
        nc, in_maps, core_ids=list(range(NCORES)), trace=trace, **spmd_kwargs
    )
    # results[c]["yt"] is y[:, c*NS:(c+1)*NS].T — stack to y.T then transpose
    y_t = np.concatenate([res.results[c]["yt"] for c in range(NCORES)], axis=0)
    out = np.ascontiguousarray(y_t.T).reshape(B, S, D)
    return out, res


def kernel(**inputs) -> np.ndarray:
    out, _ = _run(inputs)
    return out
